# revision 40
# baseline (speedup 1.0000x reference)
"""EventDrivenODECell Trainium2 kernel.

Math (reference semantics):
  dt = (t_end - t_start)/5
  5 Euler steps: h += dt * (W3 tanh(W2 tanh(W1a h + [bd1 + W1b te(t)])) + bd3)
    where te(t) depends only on the scalar t -> folded on host into a
    per-step bias  b1s = bd1 + W1b @ te(t_s);  dt folded into W3/bd3.
  event: out = h + sigmoid(Wg ef + bg) * (We2 relu(We1h h + We1e ef + be1) + be2)

Algebraic refactor (one matmul layer per Euler step removed): track the
layer-1 pre-activation  pre1_s = W1a h_s + b1s(s)  instead of h:
  pre1_{s+1} = pre1_s + [b1s(s+1)-b1s(s)] + (W1a W3') z2_s      (W13 on host)
  h_5        = h_0 + W3' (sum_s z2_s) + 5 b3'                   (once, at end)
so each step needs only the W13 and W2 matmuls; h_0 stays resident and the
W3 apply happens once. Validated vs the reference: rel err 5.8e-4.

Device layout: feature-major activations [feat, batch]; batch sharded 8 ways
(8192 rows/core), 8 column-chunks of 1024 rows, grouped 4 per ODE sweep.
PSUM tiles are [128, 2, 512] f32 (2 banks), pool depth 4 — a pair-wide
(4-bank, depth-2) PSUM variant lock-steps fills against drains and loses
~10us, so drains stay chunk-granular except z1. pre1 lives in 2-chunk pair
tiles so z1 = tanh(pre1) runs as [128, 2048] SBUF->SBUF ACT ops (ACT costs
(N+352)/1.2 ns; wider ops cut the fixed overhead — ACT and DVE pace the
refactored ODE, the PE no longer does). The step-0 z2 activation writes the
Z accumulator tile directly (a separate copy pass cost ~44us across
DVE/GpSimd); later steps accumulate into fresh pooled tiles (non-aliased
f16 SBUF tensor_tensor ops are eligible for the DVE 2x path; in-place
updates are not), split DVE (m0) / GpSimd (m1). The event phase per group
is software-pipelined per chunk (we2+epilogue lag one chunk) with drains
split across ACT/DVE; epilogue intermediates and outT are f16 and the host
upcasts. 16 junk matmuls warm the PE clock (HAM needs ~3.4us of dense
matmul activity before the 2.4GHz flip).
"""

import os
import sys

sys.path.insert(0, "/opt/trn_rl_repo")

import numpy as np

import concourse.bacc as bacc
import concourse.mybir as mybir
import concourse.tile as tile
from concourse.bass_utils import run_bass_kernel_spmd

B = 65536
HID = 256
EVT = 64
TEMB = 32
NUM_STEPS = 5
N_CORES = 8
R = B // N_CORES          # rows per core
S = 512                   # matmul moving-dim / PSUM bank quantum
CHUNK = 1024              # rows per processing chunk (= 2 PSUM banks)
NS = CHUNK // S           # N-splits per chunk
N_CHUNKS = R // CHUNK     # 8
GROUP = 8

MODE = os.environ.get("KMODE", "f16")   # "f16" | "f32r" | "f32"

f32 = mybir.dt.float32
f32r = mybir.dt.float32r
f16 = mybir.dt.float16

_CACHE = {}

# bias-pack column indices
COL_B1S = 0          # col 0: b1s(0); cols 1..4: b1s(s) - b1s(s-1)
COL_B2 = 5
COL_B3 = 6           # 5 * dt * bd3 (single final h apply)
COL_BE1 = 7
COL_BE2 = 8
COL_BG = 9
N_BIAS_COLS = 10


def _build(mode):
    wdt = {"f32r": f32r, "f32": f32, "f16": f16}[mode]
    nc = bacc.Bacc("TRN2", target_bir_lowering=False, debug=False,
                   num_devices=N_CORES)

    hT_d = nc.dram_tensor("hT", [HID, R], wdt, kind="ExternalInput")
    efT_d = nc.dram_tensor("efT", [EVT, R], wdt, kind="ExternalInput")
    w1_d = nc.dram_tensor("w1", [HID, HID], wdt, kind="ExternalInput")
    w2_d = nc.dram_tensor("w2", [HID, HID], wdt, kind="ExternalInput")
    w3_d = nc.dram_tensor("w3", [HID, HID], wdt, kind="ExternalInput")
    w13_d = nc.dram_tensor("w13", [HID, HID], wdt, kind="ExternalInput")
    we1h_d = nc.dram_tensor("we1h", [HID, HID], wdt, kind="ExternalInput")
    we1e_d = nc.dram_tensor("we1e", [EVT, HID], wdt, kind="ExternalInput")
    we2_d = nc.dram_tensor("we2", [HID, HID], wdt, kind="ExternalInput")
    wg_d = nc.dram_tensor("wg", [EVT, HID], wdt, kind="ExternalInput")
    biasp_d = nc.dram_tensor("biasp", [HID, N_BIAS_COLS], f32,
                             kind="ExternalInput")
    outT_d = nc.dram_tensor("outT", [HID, R], f16, kind="ExternalOutput")

    Tanh = mybir.ActivationFunctionType.Tanh
    Sigmoid = mybir.ActivationFunctionType.Sigmoid
    Relu = mybir.ActivationFunctionType.Relu
    Identity = mybir.ActivationFunctionType.Identity
    add = mybir.AluOpType.add
    mult = mybir.AluOpType.mult

    with tile.TileContext(nc) as tc:
        with (
            tc.tile_pool(name="consts", bufs=1) as consts,
            tc.tile_pool(name="h", bufs=1) as h_pool,
            tc.tile_pool(name="ef", bufs=1) as ef_pool,
            tc.tile_pool(name="pre1", bufs=1) as pre1_pool,
            tc.tile_pool(name="zacc", bufs=1) as zacc_pool,
            tc.tile_pool(name="z1", bufs=6) as z1_pool,
            tc.tile_pool(name="u1", bufs=5) as u1_pool,
            tc.tile_pool(name="z2", bufs=8) as z2_pool,
            tc.tile_pool(name="stage", bufs=4) as stage_pool,
            tc.tile_pool(name="psum", bufs=4, space="PSUM") as psum_pool,
        ):
            # ---- input DMAs. Two HWDGE queues issue descriptors in
            # parallel: first-needed (w1, h0, h1) on the otherwise-idle ACT
            # queue; the rest trickled on Sync in use order (~624ns of
            # issue time per descriptor). ----
            def load_w(d, name, kparts, kdim=128, eng=None):
                eng = eng or nc.sync
                ts = []
                for k in range(kparts):
                    t = consts.tile([kdim, HID], wdt, tag=f"{name}{k}",
                                    name=f"{name}{k}")
                    eng.dma_start(t[:], d.ap()[k * kdim:(k + 1) * kdim, :])
                    ts.append(t)
                return ts

            h = [[h_pool.tile([128, NS, S], wdt, tag=f"h{c}_{m}",
                              name=f"h{c}_{m}")
                  for m in range(2)] for c in range(N_CHUNKS)]

            def load_h(c, eng=None):
                eng = eng or nc.sync
                for m in range(2):
                    eng.dma_start(
                        h[c][m][:],
                        hT_d.ap()[m * 128:(m + 1) * 128,
                                  c * CHUNK:(c + 1) * CHUNK])

            w1 = load_w(w1_d, "w1", 2, eng=nc.scalar)
            load_h(0, eng=nc.scalar)
            load_h(1, eng=nc.scalar)
            biasp = []
            for m in range(2):
                t = consts.tile([128, N_BIAS_COLS], f32, tag=f"biasp{m}",
                                name=f"biasp{m}")
                nc.sync.dma_start(t[:], biasp_d.ap()[m * 128:(m + 1) * 128, :])
                biasp.append(t)
            load_h(2)
            load_h(3)
            w2 = load_w(w2_d, "w2", 2)
            w13 = load_w(w13_d, "w13", 2)
            load_h(4)
            load_h(5)
            w3 = load_w(w3_d, "w3", 2)
            load_h(6)
            load_h(7)
            we1h = load_w(we1h_d, "we1h", 2)
            we2 = load_w(we2_d, "we2", 2)
            # EVT-dim weights live in both partition halves so the two
            # m-half K=64 matmuls can run on distinct PE row groups.
            def load_evt_w(d, name):
                t = consts.tile([128, HID], wdt, tag=name, name=name)
                nc.sync.dma_start(t[0:EVT, :], d.ap())
                nc.sync.dma_start(t[EVT:128, :], d.ap())
                return t

            we1e = load_evt_w(we1e_d, "we1e")   # [128, 256], duplicated rows
            wg = load_evt_w(wg_d, "wg")
            # event features, duplicated into both partition halves, all
            # chunks in one resident tile (2 descriptors, needed ~200us in)
            efall = ef_pool.tile([128, N_CHUNKS, NS, S], wdt, tag="efall",
                                 name="efall")
            for half in range(2):
                nc.sync.dma_start(
                    efall[half * EVT:(half + 1) * EVT], efT_d.ap())

            def bcol(m, col):
                return biasp[m][:, col:col + 1]

            def efap(c, m, j):
                return efall[m * EVT:(m + 1) * EVT, c, j]

            # pre1 quad tiles: [128, 4 chunks, NS, S] per (quad, m) so the
            # tanh runs as one 4096-wide SBUF op ((4096+352)/1.2 = 3.71us
            # vs 2x2.0us for pairs: ACT is the ODE pacing engine, so the
            # per-op overhead saving is a direct span saving).
            pre1 = [[pre1_pool.tile([128, 4, NS, S], f16, tag=f"p{q}_{m}",
                                    name=f"p{q}_{m}") for m in range(2)]
                    for q in range(N_CHUNKS // 4)]

            def pre1_slice(c, m):
                return pre1[c // 4][m][:, c % 4]

            # ---- PE warmup: dependency-free junk matmuls ramp HAM to the
            # full clock while the first h/w DMAs land ----
            warm = consts.tile([128, S], wdt, tag="warm", name="warm")
            nc.vector.memset(warm[:], 0.0)
            wps = psum_pool.tile([128, S], f32, tag="ps", name="wps")
            for _ in range(10):
                nc.tensor.matmul(wps[:], warm[:, :128], warm[:],
                                 start=True, stop=True)
            # prefetch ACT function tables while ACT is idle
            wz = stage_pool.tile([128, S], f16, tag="st", name="wz")
            nc.scalar.activation(wz[:], warm[:], Sigmoid)
            nc.scalar.activation(wz[:], warm[:], Tanh)

            def mm_chunk(ps, win, xs, m, kparts=2):
                """ps [128,NS,S] (PSUM) += win[k][:, m-blk].T @ xs(k, j)."""
                for k in range(kparts):
                    wblk = win[k][:, m * 128:(m + 1) * 128]
                    for j in range(NS):
                        nc.tensor.matmul(ps[:, j], wblk, xs(k, j),
                                         start=(k == 0),
                                         stop=(k == kparts - 1))

            groups = [list(range(g * GROUP, (g + 1) * GROUP))
                      for g in range(N_CHUNKS // GROUP)]

            # ---------------- event-phase helpers ----------------
            gates = {}
            u1s = {}
            psus = {}

            # Z accumulator tiles, resident per (chunk, m); written directly
            # by the step-0 z2 activation, accumulated in place after
            Zt = [[zacc_pool.tile([128, NS, S], f16, tag=f"Z{c}_{m}",
                                  name=f"Z{c}_{m}") for m in range(2)]
                  for c in range(N_CHUNKS)]

            def h5_apply(c):
                # h <- h0 + W3' Z + 5 b3'  (in place; the old h tile is h0)
                for m in range(2):
                    ps = psum_pool.tile([128, NS, S], f32, tag="ps",
                                        name=f"ph5{m}")
                    mm_chunk(ps, w3, lambda k, j: Zt[c][k][:, j], m)
                    nc.vector.scalar_tensor_tensor(
                        h[c][m][:], ps[:], bcol(m, COL_B3),
                        h[c][m][:], op0=add, op1=add)

            def gate_mms(c):
                psg = [psum_pool.tile([128, NS, S], f32, tag="ps",
                                      name=f"psg{m}") for m in range(2)]
                for j in range(NS):
                    for m in range(2):
                        nc.tensor.matmul(
                            psg[m][:, j],
                            wg[m * EVT:(m + 1) * EVT,
                               m * 128:(m + 1) * 128],
                            efap(c, m, j),
                            start=True, stop=True,
                            tile_position=(64 * m, 0))
                gs = []
                for m in range(2):
                    gate = z2_pool.tile([128, NS, S], f16, tag="z",
                                        name=f"g{c}_{m}")
                    nc.scalar.activation(gate[:], psg[m][:], Sigmoid,
                                         bias=bcol(m, COL_BG))
                    gs.append(gate)
                gates[c] = gs

            def u1h_mms(c):
                psu = [psum_pool.tile([128, NS, S], f32, tag="ps",
                                      name=f"psu{m}") for m in range(2)]
                for m in range(2):
                    for k in range(2):
                        wblk = we1h[k][:, m * 128:(m + 1) * 128]
                        for j in range(NS):
                            nc.tensor.matmul(psu[m][:, j], wblk,
                                             h[c][k][:, j],
                                             start=(k == 0), stop=False)
                psus[c] = psu

            def u1e_mms(c):
                psu = psus[c]
                for j in range(NS):
                    for m in range(2):
                        nc.tensor.matmul(
                            psu[m][:, j],
                            we1e[m * EVT:(m + 1) * EVT,
                                 m * 128:(m + 1) * 128],
                            efap(c, m, j),
                            start=False, stop=True,
                            tile_position=(64 * m, 0))
                ts = []
                for m in range(2):
                    o = u1_pool.tile([128, NS, S], wdt, tag="u",
                                     name=f"u{c}_{m}")
                    nc.scalar.activation(o[:], psu[m][:], Relu,
                                         bias=bcol(m, COL_BE1))
                    ts.append(o)
                u1s[c] = ts

            def we2_epilogue(c, last):
                for m in range(2):
                    psp = psum_pool.tile([128, NS, S], f32, tag="ps",
                                         name=f"psp{m}")
                    mm_chunk(psp, we2, lambda k, j: u1s[c][k][:, j], m)
                    # tmp = (psum_upd + be2) * gate; out = tmp + h5 (f16,
                    # DVE fast path); m=1 adds on the otherwise-idle GpSimd
                    # except the last chunk. The last chunk runs per-j so
                    # its post-matmul STT->add->DMA tail chain is half as
                    # deep per stage.
                    tmp = z2_pool.tile([128, NS, S], f16, tag="z",
                                       name=f"t{c}_{m}")
                    stg = stage_pool.tile([128, NS, S], f16, tag="st",
                                          name=f"s{c}_{m}")
                    jsplits = [slice(j, j + 1) for j in range(NS)] if last \
                        else [slice(0, NS)]
                    for js in jsplits:
                        nc.vector.scalar_tensor_tensor(
                            tmp[:, js], psp[:, js], bcol(m, COL_BE2),
                            gates[c][m][:, js], op0=add, op1=mult)
                        # DVE paces the event (~6.35us/chunk vs PE 6.2):
                        # keep only the odd chunks' m0 add on DVE
                        eng = (nc.gpsimd
                               if (not last and (m == 1 or c % 2 == 0))
                               else nc.vector)
                        eng.tensor_add(stg[:, js], tmp[:, js],
                                       h[c][m][:, js])
                        nc.sync.dma_start(
                            outT_d.ap()[m * 128:(m + 1) * 128,
                                        c * CHUNK + js.start * S:
                                        c * CHUNK + js.stop * S],
                            stg[:, js])

            # ---------------- main schedule ----------------
            def z1_quad(q):
                zz = []
                for m in range(2):
                    o = z1_pool.tile([128, 4, NS, S], f16, tag="z1",
                                     name=f"z1_{q}_{m}")
                    nc.scalar.activation(o[:], pre1[q][m][:], Tanh)
                    zz.append(o)
                return zz

            for gi, chunks in enumerate(groups):
                # init: pre1 = W1a h0 + b1s(0); m-halves drain via DVE/ACT.
                # The step-0 z1 tanh for each pair is emitted as soon as its
                # two chunks' inits are done — emitting them after ALL inits
                # queues them behind 8 Identity drains on the in-order ACT
                # queue, stalling the PE long enough to re-throttle HAM.
                z1p0 = {}
                for c in chunks:
                    for m in range(2):
                        ps = psum_pool.tile([128, NS, S], f32, tag="ps",
                                            name=f"pi{m}")
                        mm_chunk(ps, w1, lambda k, j: h[c][k][:, j], m)
                        # most init drains go to DVE: ACT runs ~94% busy in
                        # the early ODE and every Identity drain queued
                        # there delays the step-0/1 tanh chain
                        if m == 0 or c % 2 == 1:
                            nc.vector.tensor_scalar_add(
                                pre1_slice(c, m)[:], ps[:],
                                bcol(m, COL_B1S))
                        else:
                            nc.scalar.activation(pre1_slice(c, m)[:], ps[:],
                                                 Identity,
                                                 bias=bcol(m, COL_B1S))
                    if c % 4 == 3:
                        z1p0[c // 4] = z1_quad(c // 4)
                for s in range(NUM_STEPS):
                    # z1 = tanh(pre1): 2048-wide SBUF->SBUF ACT per pair.
                    # (Emitting step s+1's z1 ops inside step s's chunk loop
                    # was tried and is WORSE: it displaces z2 drains on the
                    # saturated ACT queue. The ODE runs at its ACT floor,
                    # ~34.4us of ACT work per step vs 27.3us of PE work, so
                    # the step-boundary PE gap is unavoidable slack.)
                    if s == 0:
                        z1p = z1p0
                    else:
                        z1p = {}
                        for qi_ in range(len(chunks) // 4):
                            q = chunks[0] // 4 + qi_
                            z1p[q] = z1_quad(q)
                    # z2 = tanh(W2 z1 + b2) per chunk, with the previous
                    # chunk's Z-add / W13 / pre1-update interleaved so
                    # ACT-drained and DVE-drained PSUM tiles alternate and
                    # the z2 live-set stays small. Step 0 writes the Z
                    # accumulator tiles directly (no copy pass).
                    z2s = {}

                    def zupd(c):
                        for m in range(2):
                            if s > 0:
                                # m1 and half of m0 on GpSimd: DVE carries
                                # the pre1 STTs and would otherwise pace
                                # the late-ODE region at ~88% busy
                                eng = (nc.vector if (m == 0 and c % 2 == 0)
                                       else nc.gpsimd)
                                eng.tensor_add(Zt[c][m][:], Zt[c][m][:],
                                               z2s[c][m][:])
                        if s < NUM_STEPS - 1:
                            for m in range(2):
                                ps = psum_pool.tile([128, NS, S], f32,
                                                    tag="ps", name=f"pu{m}")
                                mm_chunk(ps, w13,
                                         lambda k, j: z2s[c][k][:, j], m)
                                nc.vector.scalar_tensor_tensor(
                                    pre1_slice(c, m)[:], ps[:],
                                    bcol(m, COL_B1S + s + 1),
                                    pre1_slice(c, m)[:], op0=add, op1=add)

                    for c in chunks:
                        zz = []
                        for m in range(2):
                            ps = psum_pool.tile([128, NS, S], f32, tag="ps",
                                                name=f"p2{m}")
                            mm_chunk(
                                ps, w2, m=m,
                                xs=lambda k, j: z1p[c // 4][k][:, c % 4, j])
                            o = (Zt[c][m] if s == 0 else
                                 z2_pool.tile([128, NS, S], f16, tag="z",
                                              name=f"z2{m}"))
                            nc.scalar.activation(o[:], ps[:], Tanh,
                                                 bias=bcol(m, COL_B2))
                            zz.append(o)
                        z2s[c] = zz
                        if c > chunks[0]:
                            zupd(c - 1)
                    zupd(chunks[-1])
                # event phase for this group, pipelined per chunk with the
                # h5 apply folded in front of each chunk's event matmuls
                # h5 runs one chunk ahead of the event pipeline so the
                # u1h matmuls never wait on their own chunk's h5 STT
                last_group = gi == len(groups) - 1
                h5_apply(chunks[0])
                for c in chunks:
                    if c < chunks[-1]:
                        h5_apply(c + 1)
                    gate_mms(c)
                    u1h_mms(c)
                    if c > chunks[0]:
                        we2_epilogue(c - 1, last=False)
                    u1e_mms(c)
                we2_epilogue(chunks[-1], last=last_group)

    nc.finalize()
    return nc


def _get_nc(mode):
    if mode not in _CACHE:
        _CACHE[mode] = _build(mode)
    return _CACHE[mode]


LAST_RESULT = None


def kernel(h_prev, event_features, t_start, t_end,
           Wt1, bt1, Wt2, bt2,
           Wd1, bd1, Wd2, bd2, Wd3, bd3,
           We1, be1, We2, be2, Wg, bg):
    global LAST_RESULT
    assert h_prev.shape == (B, HID) and event_features.shape == (B, EVT)

    # ---- host-side folding (float64 for exactness, cast down once) ----
    f8 = np.float64
    dt = (f8(t_end) - f8(t_start)) / NUM_STEPS
    b1s = np.empty((NUM_STEPS, HID), dtype=f8)
    for s in range(NUM_STEPS):
        t = f8(t_start) + s * dt
        te = np.tanh(t * Wt1[:, 0].astype(f8) + bt1.astype(f8))
        te = Wt2.astype(f8) @ te + bt2.astype(f8)
        b1s[s] = bd1.astype(f8) + Wd1[:, HID:].astype(f8) @ te

    xdt = np.float16 if MODE == "f16" else np.float32
    W1a = Wd1[:, :HID].astype(f8)
    W3p = dt * Wd3.astype(f8)
    w1T = np.ascontiguousarray(W1a.T.astype(xdt))
    w2T = np.ascontiguousarray(Wd2.T, dtype=xdt)
    w3T = np.ascontiguousarray(W3p.T.astype(xdt))
    w13T = np.ascontiguousarray((W1a @ W3p).T.astype(xdt))
    we1hT = np.ascontiguousarray(We1[:, :HID].T, dtype=xdt)
    we1eT = np.ascontiguousarray(We1[:, HID:].T, dtype=xdt)
    we2T = np.ascontiguousarray(We2.T, dtype=xdt)
    wgT = np.ascontiguousarray(Wg.T, dtype=xdt)

    biasp = np.zeros((HID, N_BIAS_COLS), dtype=f8)
    biasp[:, COL_B1S] = b1s[0]
    for s in range(1, NUM_STEPS):
        biasp[:, COL_B1S + s] = b1s[s] - b1s[s - 1]
    biasp[:, COL_B2] = bd2.astype(f8)
    biasp[:, COL_B3] = NUM_STEPS * dt * bd3.astype(f8)
    biasp[:, COL_BE1] = be1.astype(f8)
    biasp[:, COL_BE2] = be2.astype(f8)
    biasp[:, COL_BG] = bg.astype(f8)
    biasp = biasp.astype(np.float32)

    hT = np.ascontiguousarray(h_prev.T, dtype=xdt)      # [HID, B]
    efT = np.ascontiguousarray(event_features.T, dtype=xdt)

    shared = dict(w1=w1T, w2=w2T, w3=w3T, w13=w13T, we1h=we1hT, we1e=we1eT,
                  we2=we2T, wg=wgT, biasp=biasp)
    in_maps = []
    for c in range(N_CORES):
        sl = slice(c * R, (c + 1) * R)
        in_maps.append(dict(
            hT=np.ascontiguousarray(hT[:, sl]),
            efT=np.ascontiguousarray(efT[:, sl]),
            **shared))

    nc = _get_nc(MODE)
    # First execution of a freshly-loaded NEFF occasionally faults the
    # exec unit (transient); retry recovers.
    last_err = None
    for _ in range(3):
        try:
            res = run_bass_kernel_spmd(nc, in_maps,
                                       core_ids=list(range(N_CORES)))
            break
        except Exception as e:  # noqa: BLE001
            last_err = e
            os.environ["BASS_NEVER_TRACE"] = "1"
            import time
            time.sleep(2)
    else:
        raise last_err
    LAST_RESULT = res

    out = np.empty((B, HID), dtype=np.float32)
    for c in range(N_CORES):
        out[c * R:(c + 1) * R, :] = res.results[c]["outT"].T.astype(np.float32)
    return out


# revision 41
# speedup vs baseline: 1.0748x; 1.0748x over previous
"""EventDrivenODECell Trainium2 kernel.

Math (reference semantics):
  dt = (t_end - t_start)/5
  5 Euler steps: h += dt * (W3 tanh(W2 tanh(W1a h + [bd1 + W1b te(t)])) + bd3)
    where te(t) depends only on the scalar t -> folded on host into a
    per-step bias  b1s = bd1 + W1b @ te(t_s);  dt folded into W3/bd3.
  event: out = h + sigmoid(Wg ef + bg) * (We2 relu(We1h h + We1e ef + be1) + be2)

Algebraic refactor (one matmul layer per Euler step removed): track the
layer-1 pre-activation  pre1_s = W1a h_s + b1s(s)  instead of h:
  pre1_{s+1} = pre1_s + [b1s(s+1)-b1s(s)] + (W1a W3') z2_s      (W13 on host)
  h_5        = h_0 + W3' (sum_s z2_s) + 5 b3'                   (once, at end)
so each step needs only the W13 and W2 matmuls; h_0 stays resident and the
W3 apply happens once. Validated vs the reference: rel err 5.8e-4.

Device layout: feature-major activations [feat, batch]; batch sharded 8 ways
(8192 rows/core), 8 column-chunks of 1024 rows, grouped 4 per ODE sweep.
PSUM tiles are [128, 2, 512] f32 (2 banks), pool depth 4 — a pair-wide
(4-bank, depth-2) PSUM variant lock-steps fills against drains and loses
~10us, so drains stay chunk-granular except z1. pre1 lives in 2-chunk pair
tiles so z1 = tanh(pre1) runs as [128, 2048] SBUF->SBUF ACT ops (ACT costs
(N+352)/1.2 ns; wider ops cut the fixed overhead — ACT and DVE pace the
refactored ODE, the PE no longer does). The step-0 z2 activation writes the
Z accumulator tile directly (a separate copy pass cost ~44us across
DVE/GpSimd); later steps accumulate into fresh pooled tiles (non-aliased
f16 SBUF tensor_tensor ops are eligible for the DVE 2x path; in-place
updates are not), split DVE (m0) / GpSimd (m1). The event phase per group
is software-pipelined per chunk (we2+epilogue lag one chunk) with drains
split across ACT/DVE; epilogue intermediates and outT are f16 and the host
upcasts. 16 junk matmuls warm the PE clock (HAM needs ~3.4us of dense
matmul activity before the 2.4GHz flip).
"""

import os
import sys

sys.path.insert(0, "/opt/trn_rl_repo")

import numpy as np

import concourse.bacc as bacc
import concourse.mybir as mybir
import concourse.tile as tile
from concourse.bass_utils import run_bass_kernel_spmd

B = 65536
HID = 256
EVT = 64
TEMB = 32
NUM_STEPS = 5
N_CORES = 8
R = B // N_CORES          # rows per core
S = 512                   # matmul moving-dim / PSUM bank quantum
CHUNK = 1024              # rows per processing chunk (= 2 PSUM banks)
NS = CHUNK // S           # N-splits per chunk
N_CHUNKS = R // CHUNK     # 8
GROUP = 8

MODE = os.environ.get("KMODE", "f16")   # "f16" | "f32r" | "f32"

f32 = mybir.dt.float32
f32r = mybir.dt.float32r
f16 = mybir.dt.float16

_CACHE = {}

# bias-pack column indices
COL_B1S = 0          # col 0: b1s(0); cols 1..4: b1s(s) - b1s(s-1)
COL_B2 = 5
COL_B3 = 6           # 5 * dt * bd3 (single final h apply)
COL_BE1 = 7
COL_BE2 = 8
COL_BG = 9
N_BIAS_COLS = 10


def _build(mode):
    wdt = {"f32r": f32r, "f32": f32, "f16": f16}[mode]
    nc = bacc.Bacc("TRN2", target_bir_lowering=False, debug=False,
                   num_devices=N_CORES)

    hT_d = nc.dram_tensor("hT", [HID, R], wdt, kind="ExternalInput")
    efT_d = nc.dram_tensor("efT", [EVT, R], wdt, kind="ExternalInput")
    w1_d = nc.dram_tensor("w1", [HID, HID], wdt, kind="ExternalInput")
    w2_d = nc.dram_tensor("w2", [HID, HID], wdt, kind="ExternalInput")
    w3_d = nc.dram_tensor("w3", [HID, HID], wdt, kind="ExternalInput")
    w13_d = nc.dram_tensor("w13", [HID, HID], wdt, kind="ExternalInput")
    we1h_d = nc.dram_tensor("we1h", [HID, HID], wdt, kind="ExternalInput")
    we1e_d = nc.dram_tensor("we1e", [EVT, HID], wdt, kind="ExternalInput")
    we2_d = nc.dram_tensor("we2", [HID, HID], wdt, kind="ExternalInput")
    wg_d = nc.dram_tensor("wg", [EVT, HID], wdt, kind="ExternalInput")
    biasp_d = nc.dram_tensor("biasp", [HID, N_BIAS_COLS], f32,
                             kind="ExternalInput")
    outT_d = nc.dram_tensor("outT", [HID, R], f16, kind="ExternalOutput")

    Tanh = mybir.ActivationFunctionType.Tanh
    Sigmoid = mybir.ActivationFunctionType.Sigmoid
    Relu = mybir.ActivationFunctionType.Relu
    Identity = mybir.ActivationFunctionType.Identity
    add = mybir.AluOpType.add
    mult = mybir.AluOpType.mult

    with tile.TileContext(nc) as tc:
        with (
            tc.tile_pool(name="consts", bufs=1) as consts,
            tc.tile_pool(name="h", bufs=1) as h_pool,
            tc.tile_pool(name="ef", bufs=1) as ef_pool,
            tc.tile_pool(name="pre1", bufs=1) as pre1_pool,
            tc.tile_pool(name="zacc", bufs=1) as zacc_pool,
            tc.tile_pool(name="z1", bufs=11) as z1_pool,
            tc.tile_pool(name="u1", bufs=5) as u1_pool,
            tc.tile_pool(name="z2", bufs=8) as z2_pool,
            tc.tile_pool(name="stage", bufs=4) as stage_pool,
            tc.tile_pool(name="psum", bufs=4, space="PSUM") as psum_pool,
        ):
            # ---- input DMAs. Two HWDGE queues issue descriptors in
            # parallel: first-needed (w1, h0, h1) on the otherwise-idle ACT
            # queue; the rest trickled on Sync in use order (~624ns of
            # issue time per descriptor). ----
            def load_w(d, name, kparts, kdim=128, eng=None):
                eng = eng or nc.sync
                ts = []
                for k in range(kparts):
                    t = consts.tile([kdim, HID], wdt, tag=f"{name}{k}",
                                    name=f"{name}{k}")
                    eng.dma_start(t[:], d.ap()[k * kdim:(k + 1) * kdim, :])
                    ts.append(t)
                return ts

            h = [[h_pool.tile([128, NS, S], wdt, tag=f"h{c}_{m}",
                              name=f"h{c}_{m}")
                  for m in range(2)] for c in range(N_CHUNKS)]

            def load_h(c, eng=None):
                eng = eng or nc.sync
                for m in range(2):
                    eng.dma_start(
                        h[c][m][:],
                        hT_d.ap()[m * 128:(m + 1) * 128,
                                  c * CHUNK:(c + 1) * CHUNK])

            w1 = load_w(w1_d, "w1", 2, eng=nc.scalar)
            load_h(0, eng=nc.scalar)
            load_h(1, eng=nc.scalar)
            biasp = []
            for m in range(2):
                t = consts.tile([128, N_BIAS_COLS], f32, tag=f"biasp{m}",
                                name=f"biasp{m}")
                nc.sync.dma_start(t[:], biasp_d.ap()[m * 128:(m + 1) * 128, :])
                biasp.append(t)
            load_h(2)
            load_h(3)
            w2 = load_w(w2_d, "w2", 2)
            w13 = load_w(w13_d, "w13", 2)
            load_h(4)
            load_h(5)
            w3 = load_w(w3_d, "w3", 2)
            load_h(6)
            load_h(7)
            we1h = load_w(we1h_d, "we1h", 2)
            we2 = load_w(we2_d, "we2", 2)
            # EVT-dim weights live in both partition halves so the two
            # m-half K=64 matmuls can run on distinct PE row groups.
            def load_evt_w(d, name):
                t = consts.tile([128, HID], wdt, tag=name, name=name)
                nc.sync.dma_start(t[0:EVT, :], d.ap())
                nc.sync.dma_start(t[EVT:128, :], d.ap())
                return t

            we1e = load_evt_w(we1e_d, "we1e")   # [128, 256], duplicated rows
            wg = load_evt_w(wg_d, "wg")
            # event features, duplicated into both partition halves, all
            # chunks in one resident tile (2 descriptors, needed ~200us in)
            efall = ef_pool.tile([128, N_CHUNKS, NS, S], wdt, tag="efall",
                                 name="efall")
            for half in range(2):
                nc.sync.dma_start(
                    efall[half * EVT:(half + 1) * EVT], efT_d.ap())

            def bcol(m, col):
                return biasp[m][:, col:col + 1]

            def efap(c, m, j):
                return efall[m * EVT:(m + 1) * EVT, c, j]

            # pre1 pair tiles: [128, 2 chunks, NS, S] per (group-pair, m) so
            # the tanh runs as one 2048-wide SBUF op.
            pre1 = [[pre1_pool.tile([128, 2, NS, S], f16, tag=f"p{p}_{m}",
                                    name=f"p{p}_{m}") for m in range(2)]
                    for p in range(N_CHUNKS // 2)]

            def pre1_slice(c, m):
                return pre1[c // 2][m][:, c % 2]

            # ---- PE warmup: dependency-free junk matmuls ramp HAM to the
            # full clock while the first h/w DMAs land ----
            warm = consts.tile([128, S], wdt, tag="warm", name="warm")
            nc.vector.memset(warm[:], 0.0)
            wps = psum_pool.tile([128, S], f32, tag="ps", name="wps")
            for _ in range(10):
                nc.tensor.matmul(wps[:], warm[:, :128], warm[:],
                                 start=True, stop=True)
            # prefetch ACT function tables while ACT is idle
            wz = stage_pool.tile([128, S], f16, tag="st", name="wz")
            nc.scalar.activation(wz[:], warm[:], Sigmoid)
            nc.scalar.activation(wz[:], warm[:], Tanh)

            def mm_chunk(ps, win, xs, m, kparts=2):
                """ps [128,NS,S] (PSUM) += win[k][:, m-blk].T @ xs(k, j)."""
                for k in range(kparts):
                    wblk = win[k][:, m * 128:(m + 1) * 128]
                    for j in range(NS):
                        nc.tensor.matmul(ps[:, j], wblk, xs(k, j),
                                         start=(k == 0),
                                         stop=(k == kparts - 1))

            groups = [list(range(g * GROUP, (g + 1) * GROUP))
                      for g in range(N_CHUNKS // GROUP)]

            # ---------------- event-phase helpers ----------------
            gates = {}
            u1s = {}
            psus = {}

            # Z accumulator tiles, resident per (chunk, m); written directly
            # by the step-0 z2 activation, accumulated in place after
            Zt = [[zacc_pool.tile([128, NS, S], f16, tag=f"Z{c}_{m}",
                                  name=f"Z{c}_{m}") for m in range(2)]
                  for c in range(N_CHUNKS)]

            def h5_apply(c):
                # h <- h0 + W3' Z + 5 b3'  (in place; the old h tile is h0)
                for m in range(2):
                    ps = psum_pool.tile([128, NS, S], f32, tag="ps",
                                        name=f"ph5{m}")
                    mm_chunk(ps, w3, lambda k, j: Zt[c][k][:, j], m)
                    nc.vector.scalar_tensor_tensor(
                        h[c][m][:], ps[:], bcol(m, COL_B3),
                        h[c][m][:], op0=add, op1=add)

            def gate_mms(c):
                psg = [psum_pool.tile([128, NS, S], f32, tag="ps",
                                      name=f"psg{m}") for m in range(2)]
                for j in range(NS):
                    for m in range(2):
                        nc.tensor.matmul(
                            psg[m][:, j],
                            wg[m * EVT:(m + 1) * EVT,
                               m * 128:(m + 1) * 128],
                            efap(c, m, j),
                            start=True, stop=True,
                            tile_position=(64 * m, 0))
                gs = []
                for m in range(2):
                    gate = z2_pool.tile([128, NS, S], f16, tag="z",
                                        name=f"g{c}_{m}")
                    nc.scalar.activation(gate[:], psg[m][:], Sigmoid,
                                         bias=bcol(m, COL_BG))
                    gs.append(gate)
                gates[c] = gs

            def u1h_mms(c):
                psu = [psum_pool.tile([128, NS, S], f32, tag="ps",
                                      name=f"psu{m}") for m in range(2)]
                for m in range(2):
                    for k in range(2):
                        wblk = we1h[k][:, m * 128:(m + 1) * 128]
                        for j in range(NS):
                            nc.tensor.matmul(psu[m][:, j], wblk,
                                             h[c][k][:, j],
                                             start=(k == 0), stop=False)
                psus[c] = psu

            def u1e_mms(c):
                psu = psus[c]
                for j in range(NS):
                    for m in range(2):
                        nc.tensor.matmul(
                            psu[m][:, j],
                            we1e[m * EVT:(m + 1) * EVT,
                                 m * 128:(m + 1) * 128],
                            efap(c, m, j),
                            start=False, stop=True,
                            tile_position=(64 * m, 0))
                ts = []
                for m in range(2):
                    o = u1_pool.tile([128, NS, S], wdt, tag="u",
                                     name=f"u{c}_{m}")
                    nc.scalar.activation(o[:], psu[m][:], Relu,
                                         bias=bcol(m, COL_BE1))
                    ts.append(o)
                u1s[c] = ts

            def we2_epilogue(c, last):
                for m in range(2):
                    psp = psum_pool.tile([128, NS, S], f32, tag="ps",
                                         name=f"psp{m}")
                    mm_chunk(psp, we2, lambda k, j: u1s[c][k][:, j], m)
                    # tmp = (psum_upd + be2) * gate; out = tmp + h5 (f16,
                    # DVE fast path); m=1 adds on the otherwise-idle GpSimd
                    # except the last chunk. The last chunk runs per-j so
                    # its post-matmul STT->add->DMA tail chain is half as
                    # deep per stage.
                    tmp = z2_pool.tile([128, NS, S], f16, tag="z",
                                       name=f"t{c}_{m}")
                    stg = stage_pool.tile([128, NS, S], f16, tag="st",
                                          name=f"s{c}_{m}")
                    jsplits = [slice(j, j + 1) for j in range(NS)] if last \
                        else [slice(0, NS)]
                    for js in jsplits:
                        nc.vector.scalar_tensor_tensor(
                            tmp[:, js], psp[:, js], bcol(m, COL_BE2),
                            gates[c][m][:, js], op0=add, op1=mult)
                        # DVE paces the event (~6.35us/chunk vs PE 6.2):
                        # keep only the odd chunks' m0 add on DVE
                        eng = (nc.gpsimd
                               if (not last and (m == 1 or c % 2 == 0))
                               else nc.vector)
                        eng.tensor_add(stg[:, js], tmp[:, js],
                                       h[c][m][:, js])
                        nc.sync.dma_start(
                            outT_d.ap()[m * 128:(m + 1) * 128,
                                        c * CHUNK + js.start * S:
                                        c * CHUNK + js.stop * S],
                            stg[:, js])

            # ---------------- main schedule ----------------
            def z1_pair(p):
                zz = []
                for m in range(2):
                    o = z1_pool.tile([128, 2, NS, S], f16, tag="z1",
                                     name=f"z1_{p}_{m}")
                    nc.scalar.activation(o[:], pre1[p][m][:], Tanh)
                    zz.append(o)
                return zz

            for gi, chunks in enumerate(groups):
                # init: pre1 = W1a h0 + b1s(0); m-halves drain via DVE/ACT.
                # The step-0 z1 tanh for each pair is emitted as soon as its
                # two chunks' inits are done — emitting them after ALL inits
                # queues them behind 8 Identity drains on the in-order ACT
                # queue, stalling the PE long enough to re-throttle HAM.
                z1p0 = {}
                for c in chunks:
                    for m in range(2):
                        ps = psum_pool.tile([128, NS, S], f32, tag="ps",
                                            name=f"pi{m}")
                        mm_chunk(ps, w1, lambda k, j: h[c][k][:, j], m)
                        # most init drains go to DVE: ACT runs ~94% busy in
                        # the early ODE and every Identity drain queued
                        # there delays the step-0/1 tanh chain
                        if m == 0 or c % 2 == 1:
                            nc.vector.tensor_scalar_add(
                                pre1_slice(c, m)[:], ps[:],
                                bcol(m, COL_B1S))
                        else:
                            nc.scalar.activation(pre1_slice(c, m)[:], ps[:],
                                                 Identity,
                                                 bias=bcol(m, COL_B1S))
                    if c % 2 == 1:
                        z1p0[c // 2] = z1_pair(c // 2)
                for s in range(NUM_STEPS):
                    # z1 = tanh(pre1): 2048-wide SBUF->SBUF ACT per pair.
                    # (Emitting step s+1's z1 ops inside step s's chunk loop
                    # was tried and is WORSE: it displaces z2 drains on the
                    # saturated ACT queue. The ODE runs at its ACT floor,
                    # ~34.4us of ACT work per step vs 27.3us of PE work, so
                    # the step-boundary PE gap is unavoidable slack.)
                    if s == 0:
                        z1p = z1p0
                    else:
                        z1p = {}
                        for pi_ in range(len(chunks) // 2):
                            p = chunks[0] // 2 + pi_
                            z1p[p] = z1_pair(p)
                    # z2 = tanh(W2 z1 + b2) per chunk, with the previous
                    # chunk's Z-add / W13 / pre1-update interleaved so
                    # ACT-drained and DVE-drained PSUM tiles alternate and
                    # the z2 live-set stays small. Step 0 writes the Z
                    # accumulator tiles directly (no copy pass).
                    z2s = {}

                    def zupd(c):
                        for m in range(2):
                            if s > 0:
                                # m1 and half of m0 on GpSimd: DVE carries
                                # the pre1 STTs and would otherwise pace
                                # the late-ODE region at ~88% busy
                                eng = (nc.vector if (m == 0 and c % 2 == 0)
                                       else nc.gpsimd)
                                eng.tensor_add(Zt[c][m][:], Zt[c][m][:],
                                               z2s[c][m][:])
                        if s < NUM_STEPS - 1:
                            for m in range(2):
                                ps = psum_pool.tile([128, NS, S], f32,
                                                    tag="ps", name=f"pu{m}")
                                mm_chunk(ps, w13,
                                         lambda k, j: z2s[c][k][:, j], m)
                                nc.vector.scalar_tensor_tensor(
                                    pre1_slice(c, m)[:], ps[:],
                                    bcol(m, COL_B1S + s + 1),
                                    pre1_slice(c, m)[:], op0=add, op1=add)

                    for c in chunks:
                        zz = []
                        for m in range(2):
                            ps = psum_pool.tile([128, NS, S], f32, tag="ps",
                                                name=f"p2{m}")
                            mm_chunk(
                                ps, w2, m=m,
                                xs=lambda k, j: z1p[c // 2][k][:, c % 2, j])
                            o = (Zt[c][m] if s == 0 else
                                 z2_pool.tile([128, NS, S], f16, tag="z",
                                              name=f"z2{m}"))
                            nc.scalar.activation(o[:], ps[:], Tanh,
                                                 bias=bcol(m, COL_B2))
                            zz.append(o)
                        z2s[c] = zz
                        if c > chunks[0]:
                            zupd(c - 1)
                    zupd(chunks[-1])
                # event phase for this group, pipelined per chunk with the
                # h5 apply folded in front of each chunk's event matmuls
                # h5 runs one chunk ahead of the event pipeline so the
                # u1h matmuls never wait on their own chunk's h5 STT
                last_group = gi == len(groups) - 1
                h5_apply(chunks[0])
                for c in chunks:
                    if c < chunks[-1]:
                        h5_apply(c + 1)
                    gate_mms(c)
                    u1h_mms(c)
                    if c > chunks[0]:
                        we2_epilogue(c - 1, last=False)
                    u1e_mms(c)
                we2_epilogue(chunks[-1], last=last_group)

    nc.finalize()
    return nc


def _get_nc(mode):
    if mode not in _CACHE:
        _CACHE[mode] = _build(mode)
    return _CACHE[mode]


LAST_RESULT = None


def kernel(h_prev, event_features, t_start, t_end,
           Wt1, bt1, Wt2, bt2,
           Wd1, bd1, Wd2, bd2, Wd3, bd3,
           We1, be1, We2, be2, Wg, bg):
    global LAST_RESULT
    assert h_prev.shape == (B, HID) and event_features.shape == (B, EVT)

    # ---- host-side folding (float64 for exactness, cast down once) ----
    f8 = np.float64
    dt = (f8(t_end) - f8(t_start)) / NUM_STEPS
    b1s = np.empty((NUM_STEPS, HID), dtype=f8)
    for s in range(NUM_STEPS):
        t = f8(t_start) + s * dt
        te = np.tanh(t * Wt1[:, 0].astype(f8) + bt1.astype(f8))
        te = Wt2.astype(f8) @ te + bt2.astype(f8)
        b1s[s] = bd1.astype(f8) + Wd1[:, HID:].astype(f8) @ te

    xdt = np.float16 if MODE == "f16" else np.float32
    W1a = Wd1[:, :HID].astype(f8)
    W3p = dt * Wd3.astype(f8)
    w1T = np.ascontiguousarray(W1a.T.astype(xdt))
    w2T = np.ascontiguousarray(Wd2.T, dtype=xdt)
    w3T = np.ascontiguousarray(W3p.T.astype(xdt))
    w13T = np.ascontiguousarray((W1a @ W3p).T.astype(xdt))
    we1hT = np.ascontiguousarray(We1[:, :HID].T, dtype=xdt)
    we1eT = np.ascontiguousarray(We1[:, HID:].T, dtype=xdt)
    we2T = np.ascontiguousarray(We2.T, dtype=xdt)
    wgT = np.ascontiguousarray(Wg.T, dtype=xdt)

    biasp = np.zeros((HID, N_BIAS_COLS), dtype=f8)
    biasp[:, COL_B1S] = b1s[0]
    for s in range(1, NUM_STEPS):
        biasp[:, COL_B1S + s] = b1s[s] - b1s[s - 1]
    biasp[:, COL_B2] = bd2.astype(f8)
    biasp[:, COL_B3] = NUM_STEPS * dt * bd3.astype(f8)
    biasp[:, COL_BE1] = be1.astype(f8)
    biasp[:, COL_BE2] = be2.astype(f8)
    biasp[:, COL_BG] = bg.astype(f8)
    biasp = biasp.astype(np.float32)

    hT = np.ascontiguousarray(h_prev.T, dtype=xdt)      # [HID, B]
    efT = np.ascontiguousarray(event_features.T, dtype=xdt)

    shared = dict(w1=w1T, w2=w2T, w3=w3T, w13=w13T, we1h=we1hT, we1e=we1eT,
                  we2=we2T, wg=wgT, biasp=biasp)
    in_maps = []
    for c in range(N_CORES):
        sl = slice(c * R, (c + 1) * R)
        in_maps.append(dict(
            hT=np.ascontiguousarray(hT[:, sl]),
            efT=np.ascontiguousarray(efT[:, sl]),
            **shared))

    nc = _get_nc(MODE)
    # First execution of a freshly-loaded NEFF occasionally faults the
    # exec unit (transient); retry recovers.
    last_err = None
    for _ in range(3):
        try:
            res = run_bass_kernel_spmd(nc, in_maps,
                                       core_ids=list(range(N_CORES)))
            break
        except Exception as e:  # noqa: BLE001
            last_err = e
            os.environ["BASS_NEVER_TRACE"] = "1"
            import time
            time.sleep(2)
    else:
        raise last_err
    LAST_RESULT = res

    out = np.empty((B, HID), dtype=np.float32)
    for c in range(N_CORES):
        out[c * R:(c + 1) * R, :] = res.results[c]["outT"].T.astype(np.float32)
    return out
